# revision 13
# baseline (speedup 1.0000x reference)
"""DenseGATv2 Trainium2 kernel v2 (8 NeuronCores, data + sequence parallel).

Problem (hardcoded): B=4, N=1024, D=128, H=8, QKV=16, f32.
  scores[b,i,j,h] = leaky_relu(s_i[b,i,h] + s_j[b,j,h] + edge[b,i,j]*w_e[h])
  alpha = softmax_j(scores);  out = concat_h(alpha_h @ v_h) @ Wo

Sharding: core c -> batch b=c//2, query rows r0=512*(c%2) .. r0+512.
Each core returns its [512, 128] slice; host concatenates. SPMD: identical
program, host-sliced data.

v2 dataflow (scores transposed: [j=128 partitions, i=512 free] per (h, jt)):
  - host pre-splits h into bf16 hi/lo pairs and pre-converts sc to bf16;
    transposes (hT, hrT, scT) ride the DMA XBAR (dma_start_transpose),
    keeping PE/DVE out of layout work.
  - siT/sjT from bf16 hi/lo matmul triples (error ~2^-16); bc_si[h] (s_i
    broadcast along free axis) via K=1 f32r ones-matmul into PSUM.
  - per (h, jt) tile, one of two balanced paths:
      DVE-path: ONE fused custom DVE op (GAT_STT_LRELU):
        lq = max(t, 0.15*t), t = scT*we[h] + bc_si(PSUM) + sj  -> bf16
      ACT-path: z assembled fully in PSUM by PE (K=2 f32r si+sj composite
        + 4 bf16 edge matmuls vs we[h]*I), then one ACT Prelu -> bf16.
    Mix set by CFG["act_mod"] to balance ACT (which also owns Exp) vs DVE.
  - Exp once per head over [128, 4096] -> f32 eq; PV matmuls f32 with a
    ones column giving softmax denominators in PSUM for free.
  - epilogue: per-partition reciprocal + 4 tensor_scalar muls -> outN bf16.
  - final: outN blocks DMA-transposed, two Wo bf16 hi/lo matmuls per block.

Engine budget (cost model): ACT = 8 Exp (28.8us) + a Prelus; DVE = (64-a)
fused ops (659ns) + epilogue/copies; PE/DMA far below. Balance at a~16.
"""

import sys

for _p in ("/opt/trn_rl_repo",):
    if _p not in sys.path:
        sys.path.insert(0, _p)

import numpy as np

import concourse.bacc as bacc
import concourse.tile as tile
import concourse.mybir as mybir
from concourse.bass_utils import run_bass_kernel_spmd

import concourse.dve_ops as dvo
from concourse.dve_spec import Spec, Src0, Src1, C0, C1, C2, maxx, lower
from concourse.dve_uop import DveOpSpec

F32 = mybir.dt.float32
F32R = mybir.dt.float32r
BF16 = mybir.dt.bfloat16
ALU = mybir.AluOpType
ACTF = mybir.ActivationFunctionType

B, N, D, H, QKV = 4, 1024, 128, 8, 16
NEG_SLOPE = 0.15
N_CORES = 8
ROWS = 512               # query rows per core
P = 128
N_JT = N // P            # 8 key tiles
N_IC = ROWS // P         # 4 query-row chunks

CFG = {
    "act_mod": 4,        # every act_mod-th tile takes the ACT (Prelu) path
    "lq_bufs": 2,
    "eq_bufs": 2,
    "z_bufs": 1,
    "bc_bufs": 2,
    "ps_a_bufs": 1,
}

_cache = {}


def _register_gat_op():
    """Register the fused score+leaky-relu op: out = lrelu(in0*s0 + in1 + s1).

    uOp chain: t = Src0*C0 + Src1 + C1; out = max(t, t*imm2). Written into
    the per-NEFF DVE table at compile time (no firmware change); validated
    on HW (rel err at bf16 rounding level).
    """
    name = "GAT_STT_LRELU"
    for o in dvo.OPS:
        if o.name == name:
            return o
    t = Src0 * C0 + Src1 + C1
    body = maxx(t, t * C2)

    def _ref(in0, in1, s0, s1, imm2):
        tt = in0.astype(np.float32) * s0 + in1 + s1
        return np.maximum(tt, tt * imm2).astype(np.float32)

    spec = Spec(body=body, reference=_ref)
    opcode = max(dvo._SUB_OPCODE_FOR_NAME.values()) + 1
    assert opcode < 0x20
    shas = {}
    for ver in ("v3", "v4"):
        try:
            s = DveOpSpec(name=name, opcode=opcode, uops=lower(spec, ver=ver),
                          rd1_en=True)
            shas[ver] = s.sha(ver)
        except Exception:
            pass
    op = dvo.DveOp(name, spec, subdim=False, uops_sha=shas)
    dvo.OPS.append(op)
    dvo.CUSTOM_DVE_SPECS[name] = spec
    dvo._SUB_OPCODE_FOR_NAME[name] = opcode
    return op


GAT_OP = _register_gat_op()


def _build_program(local_only: int):
    nc = bacc.Bacc("TRN2", target_bir_lowering=False, debug=False)

    hhi_d = nc.dram_tensor("h_hi", [N, D], BF16, kind="ExternalInput")
    hlo_d = nc.dram_tensor("h_lo", [N, D], BF16, kind="ExternalInput")
    rhi_d = nc.dram_tensor("hr_hi", [ROWS, D], BF16, kind="ExternalInput")
    rlo_d = nc.dram_tensor("hr_lo", [ROWS, D], BF16, kind="ExternalInput")
    sc_d = nc.dram_tensor("sc_bf", [ROWS, N], BF16, kind="ExternalInput")
    waih_d = nc.dram_tensor("wai_hi", [D, H], BF16, kind="ExternalInput")
    wail_d = nc.dram_tensor("wai_lo", [D, H], BF16, kind="ExternalInput")
    wajh_d = nc.dram_tensor("waj_hi", [D, H], BF16, kind="ExternalInput")
    wajl_d = nc.dram_tensor("waj_lo", [D, H], BF16, kind="ExternalInput")
    wvh_d = nc.dram_tensor("wv_hi", [D, H * QKV], BF16, kind="ExternalInput")
    wvl_d = nc.dram_tensor("wv_lo", [D, H * QKV], BF16, kind="ExternalInput")
    woh_d = nc.dram_tensor("wo_hi", [H * QKV, D], BF16, kind="ExternalInput")
    wol_d = nc.dram_tensor("wo_lo", [H * QKV, D], BF16, kind="ExternalInput")
    we_d = nc.dram_tensor("we_row", [1, H], F32, kind="ExternalInput")
    eye_d = nc.dram_tensor("eye", [P, P], F32, kind="ExternalInput")
    ones_d = nc.dram_tensor("ones", [1, H * N], F32, kind="ExternalInput")
    out_d = nc.dram_tensor("out_rows", [ROWS, D], F32, kind="ExternalOutput")
    DBG = CFG.get("debug", False)
    if DBG:
        dbg_si_d = nc.dram_tensor("dbg_si", [H, ROWS], F32, kind="ExternalOutput")
        dbg_sj_d = nc.dram_tensor("dbg_sj", [H, N], F32, kind="ExternalOutput")
        dbg_sja_d = nc.dram_tensor("dbg_sja", [P, N_JT * H], F32, kind="ExternalOutput")
        dbg_bc_d = nc.dram_tensor("dbg_bc", [P, ROWS], F32, kind="ExternalOutput")
        dbg_lq_d = nc.dram_tensor("dbg_lq", [P, N_JT * ROWS], F32, kind="ExternalOutput")
        dbg_eq_d = nc.dram_tensor("dbg_eq", [P, N_JT * ROWS], F32, kind="ExternalOutput")
        dbg_v_d = nc.dram_tensor("dbg_v", [P, N_JT * H * (QKV + 1)], F32, kind="ExternalOutput")
        dbg_on_d = nc.dram_tensor("dbg_on", [P, N_IC * H * QKV], F32, kind="ExternalOutput")

    AM = CFG["act_mod"]

    def is_act_tile(h, jt):
        if AM <= 0:
            return False
        return (h * N_JT + jt) % AM == (AM - 1)

    with tile.TileContext(nc) as tc:
        with (
            tc.tile_pool(name="consts", bufs=1) as consts,
            tc.tile_pool(name="big", bufs=1) as big,
            tc.tile_pool(name="work", bufs=4) as work,
            tc.tile_pool(name="lqp", bufs=CFG["lq_bufs"]) as lqp,
            tc.tile_pool(name="eqp", bufs=CFG["eq_bufs"]) as eqp,
            tc.tile_pool(name="ps_a", bufs=CFG["ps_a_bufs"],
                         space="PSUM") as ps_a,
            tc.tile_pool(name="ps_z", bufs=CFG["z_bufs"], space="PSUM") as ps_z,
            tc.tile_pool(name="ps_bc", bufs=CFG["bc_bufs"], space="PSUM") as ps_bc,
            tc.tile_pool(name="ps_out", bufs=1, space="PSUM") as ps_out,
        ):
            # ---- constant / weight loads (SWDGE on pool queue: small) -----
            eye_sb = consts.tile([P, P], F32, tag="eye")
            nc.gpsimd.dma_start(out=eye_sb, in_=eye_d.ap())
            eye_bf = consts.tile([P, P], BF16, tag="eye_bf")
            nc.gpsimd.dma_start(out=eye_bf, in_=eye_d.ap())
            we_row = consts.tile([1, H], F32, tag="we_row")
            nc.gpsimd.dma_start(out=we_row, in_=we_d.ap())
            we_bc = consts.tile([P, H], F32, tag="we_bc")
            nc.gpsimd.partition_broadcast(we_bc[:], we_row[0:1, :])
            onesr = consts.tile([1, P], F32R, tag="onesr")
            nc.gpsimd.dma_start(out=onesr, in_=ones_d.ap()[0:1, 0:P])
            wai = consts.tile([P, 2, H], BF16, tag="wai")
            nc.gpsimd.dma_start(out=wai[:, 0, :], in_=waih_d.ap())
            nc.gpsimd.dma_start(out=wai[:, 1, :], in_=wail_d.ap())
            waj = consts.tile([P, 2, H], BF16, tag="waj")
            nc.gpsimd.dma_start(out=waj[:, 0, :], in_=wajh_d.ap())
            nc.gpsimd.dma_start(out=waj[:, 1, :], in_=wajl_d.ap())
            wv = consts.tile([P, 2, H * QKV], BF16, tag="wv")
            nc.gpsimd.dma_start(out=wv[:, 0, :], in_=wvh_d.ap())
            nc.gpsimd.dma_start(out=wv[:, 1, :], in_=wvl_d.ap())
            wo = consts.tile([P, 2, D], BF16, tag="wo")
            nc.gpsimd.dma_start(out=wo[:, 0, :], in_=woh_d.ap())
            nc.gpsimd.dma_start(out=wo[:, 1, :], in_=wol_d.ap())

            # ---- big loads + DMA transposes -------------------------------
            # scT (edge^T per j-tile) for the DVE-path; sc natural rows for
            # the ACT-path edge matmuls.
            scT = []
            for jt in range(N_JT):
                t = big.tile([P, ROWS], BF16, tag=f"scT_{jt}", name=f"scT_{jt}")
                (nc.sync if jt % 2 == 0 else nc.scalar).dma_start_transpose(
                    t, sc_d.ap()[:, jt * P:(jt + 1) * P])
                scT.append(t)
            sc_nat = big.tile([P, N_IC, N], BF16, tag="sc_nat")
            nc.sync.dma_start(
                out=sc_nat, in_=sc_d.ap().rearrange("(t p) j -> p t j", p=P))

            hT_hi = big.tile([P, N], BF16, tag="hT_hi")
            nc.sync.dma_start_transpose(hT_hi, hhi_d.ap())
            hT_lo = big.tile([P, N], BF16, tag="hT_lo")
            nc.scalar.dma_start_transpose(hT_lo, hlo_d.ap())
            hrT_hi = big.tile([P, ROWS], BF16, tag="hrT_hi")
            nc.sync.dma_start_transpose(hrT_hi, rhi_d.ap())
            hrT_lo = big.tile([P, ROWS], BF16, tag="hrT_lo")
            nc.scalar.dma_start_transpose(hrT_lo, rlo_d.ap())

            # ---- siT / sjT (hi/lo triple products, error ~2^-16) ----------
            ps_si = ps_a.tile([H, ROWS], F32, tag="pss", name="ps_si")
            nc.tensor.matmul(ps_si, wai[:, 0, :], hrT_hi[:, :], start=True,
                             stop=False)
            nc.tensor.matmul(ps_si, wai[:, 0, :], hrT_lo[:, :], start=False,
                             stop=False)
            nc.tensor.matmul(ps_si, wai[:, 1, :], hrT_hi[:, :], start=False,
                             stop=True)
            siT_r = consts.tile([H, ROWS], F32R, tag="siT_r")
            nc.scalar.copy(out=siT_r, in_=ps_si)
            siT_flat = consts.tile([1, H * ROWS], F32R, tag="siT_flat")
            nc.scalar.dma_start(out=siT_flat, in_=siT_r[:, :])

            sjT_sb = consts.tile([H, N], F32, tag="sjT_sb")
            for half in range(2):
                ps_sj = ps_a.tile([H, ROWS], F32, tag="pss",
                                  name=f"ps_sj_{half}")
                sl = slice(half * ROWS, (half + 1) * ROWS)
                nc.tensor.matmul(ps_sj, waj[:, 0, :], hT_hi[:, sl],
                                 start=True, stop=False)
                nc.tensor.matmul(ps_sj, waj[:, 0, :], hT_lo[:, sl],
                                 start=False, stop=False)
                nc.tensor.matmul(ps_sj, waj[:, 1, :], hT_hi[:, sl],
                                 start=False, stop=True)
                nc.scalar.copy(out=sjT_sb[:, sl], in_=ps_sj)

            # composite operands for the ACT-path z assembly:
            # lhsT2r [2, H*N]: row0 ones, row1 sjT flat
            # rhs2r  [2, H*ROWS]: row0 siT flat, row1 ones
            lhsT2r = consts.tile([2, H * N], F32R, tag="lhsT2r")
            nc.gpsimd.dma_start(out=lhsT2r[0:1, :], in_=ones_d.ap())
            nc.gpsimd.dma_start(out=lhsT2r[1:2, :], in_=sjT_sb[:, :])
            rhs2r = consts.tile([2, H * ROWS], F32R, tag="rhs2r")
            nc.gpsimd.dma_start(out=rhs2r[0:1, :], in_=siT_r[:, :])
            nc.gpsimd.dma_start(out=rhs2r[1:2, :],
                                in_=ones_d.ap()[0:1, 0:H * ROWS])

            # sj per-partition layout [P, jt, H] (custom-op scalar C1) via
            # tiny PE transposes of sjT rows.
            sj_all = consts.tile([P, N_JT, H], F32, tag="sj_all")
            for jt in range(N_JT):
                pst = ps_a.tile([P, H], F32, tag="pss", name=f"ps_sjT_{jt}")
                nc.tensor.transpose(
                    pst[:], sjT_sb[0:H, jt * P:(jt + 1) * P],
                    eye_sb[0:H, 0:H])
                nc.vector.tensor_copy(out=sj_all[:, jt, :], in_=pst)

            # v tiles: v[j, h*q] per jt from hi/lo matmuls; f32 (PV in f32,
            # denominators exact); ones column appended for the denominator.
            v_ones = big.tile([P, N_JT, H, QKV + 1], F32, tag="v_ones")
            nc.vector.memset(v_ones.rearrange("p a h q -> p (a h q)"), 1.0)
            for jt in range(N_JT):
                sl = slice(jt * P, (jt + 1) * P)
                ps_v = ps_a.tile([P, H * QKV], F32, tag="pss",
                                 name=f"ps_v_{jt}")
                nc.tensor.matmul(ps_v, hT_hi[:, sl], wv[:, 0, :], start=True,
                                 stop=False)
                nc.tensor.matmul(ps_v, hT_lo[:, sl], wv[:, 0, :], start=False,
                                 stop=False)
                nc.tensor.matmul(ps_v, hT_hi[:, sl], wv[:, 1, :], start=False,
                                 stop=True)
                nc.scalar.copy(
                    out=v_ones[:, jt, :, 0:QKV],
                    in_=ps_v.rearrange("p (h q) -> p h q", h=H))

            # eye_we per head (ACT-path edge matmul weights)
            eye_we = []
            for h in range(H):
                if AM > 0:
                    t = consts.tile([P, P], BF16, tag=f"eye_we_{h}",
                                    name=f"eye_we_{h}")
                    nc.vector.tensor_scalar(t, eye_bf, we_bc[:, h:h + 1],
                                            None, op0=ALU.mult)
                    eye_we.append(t)
                else:
                    eye_we.append(None)

            # ---- main loop ------------------------------------------------
            # separate per-ic tiles: the final XBAR dma-transpose needs its
            # source at a clean tile base (odd 256B column offsets read wrong)
            outN = [
                big.tile([P, H, QKV], BF16, tag=f"outN_{ic}",
                         name=f"outN_{ic}")
                for ic in range(N_IC)
            ]

            def emit_bc(h):
                t = ps_bc.tile([P, ROWS], F32, tag="bc", name=f"bc_{h}")
                nc.tensor.matmul(
                    t, onesr[0:1, :],
                    siT_flat[0:1, h * ROWS:(h + 1) * ROWS],
                    start=True, stop=True)
                return t

            bc_tiles = {0: emit_bc(0)}

            def emit_head_epilogue(h, po_h):
                rec4 = work.tile([P, N_IC], F32, tag="rec4")
                for ic in range(N_IC):
                    nc.vector.reciprocal(
                        out=rec4[:, ic:ic + 1],
                        in_=po_h[ic][:, QKV:QKV + 1])
                for ic in range(N_IC):
                    nc.vector.tensor_scalar_mul(
                        outN[ic][:, h, :], po_h[ic][:, 0:QKV],
                        rec4[:, ic:ic + 1])

            pending_epi = []

            for h in range(H):
                bc_h = bc_tiles.pop(h)
                po_h = [
                    ps_out.tile([P, QKV + 1], F32, tag=f"po{ic}",
                                name=f"po_{h}_{ic}")
                    for ic in range(N_IC)
                ]
                lq = lqp.tile([P, N_JT * ROWS], BF16, tag="lq")
                for jt in range(N_JT):
                    ldst = lq[:, jt * ROWS:(jt + 1) * ROWS]
                    if is_act_tile(h, jt):
                        z = ps_z.tile([P, ROWS], F32, tag="z",
                                      name=f"z_{h}_{jt}")
                        nc.tensor.matmul(
                            z, lhsT2r[:, h * N + jt * P:h * N + (jt + 1) * P],
                            rhs2r[:, h * ROWS:(h + 1) * ROWS],
                            start=True, stop=False)
                        for it in range(N_IC):
                            nc.tensor.matmul(
                                z[:, it * P:(it + 1) * P],
                                sc_nat[:, it, jt * P:(jt + 1) * P],
                                eye_we[h][:, :],
                                start=False, stop=(it == N_IC - 1))
                        nc.scalar.activation(
                            out=ldst, in_=z, func=ACTF.Prelu,
                            bias=0.0, scale=1.0, alpha=NEG_SLOPE)
                    else:
                        nc.vector._custom_dve(
                            GAT_OP, out=ldst, in0=scT[jt], in1=bc_h,
                            s0=we_bc[:, h:h + 1],
                            s1=sj_all[:, jt, h:h + 1], imm2=NEG_SLOPE)
                    if jt == 1 and pending_epi:
                        emit_head_epilogue(*pending_epi.pop())
                    if jt == 2 and h + 1 < H:
                        bc_tiles[h + 1] = emit_bc(h + 1)
                eq = eqp.tile([P, N_JT * ROWS], F32, tag="eq")
                nc.scalar.activation(out=eq, in_=lq, func=ACTF.Exp)
                for jt in range(N_JT):
                    if local_only:
                        nc.vector.tensor_tensor(
                            out=eq[:, jt * ROWS:(jt + 1) * ROWS],
                            in0=eq[:, jt * ROWS:(jt + 1) * ROWS],
                            in1=scT[jt][:, :], op=ALU.mult)
                    for ic in range(N_IC):
                        nc.tensor.matmul(
                            po_h[ic],
                            eq[:, jt * ROWS + ic * P:jt * ROWS + (ic + 1) * P],
                            v_ones[:, jt, h, :],
                            start=(jt == 0), stop=(jt == N_JT - 1))
                if DBG and h == 0:
                    d1 = big.tile([P, N_JT * ROWS], F32, tag="dbg1")
                    nc.vector.tensor_copy(out=d1, in_=lq)
                    nc.sync.dma_start(out=dbg_lq_d.ap(), in_=d1)
                    d2 = big.tile([P, N_JT * ROWS], F32, tag="dbg2")
                    nc.vector.tensor_copy(out=d2, in_=eq)
                    nc.sync.dma_start(out=dbg_eq_d.ap(), in_=d2)
                    d3 = work.tile([P, ROWS], F32, tag="dbg3")
                    nc.vector.tensor_copy(out=d3, in_=bc_h)
                    nc.sync.dma_start(out=dbg_bc_d.ap(), in_=d3)
                pending_epi.append((h, po_h))
            while pending_epi:
                emit_head_epilogue(*pending_epi.pop())
            if DBG:
                nc.gpsimd.dma_start(out=dbg_si_d.ap(), in_=siT_r[:, :])
                nc.scalar.dma_start(out=dbg_sj_d.ap(), in_=sjT_sb[:, :])
                nc.scalar.dma_start(
                    out=dbg_sja_d.ap(),
                    in_=sj_all.rearrange("p a b -> p (a b)"))
                dv = big.tile([P, N_JT * H * (QKV + 1)], F32, tag="dbgv")
                nc.vector.tensor_copy(
                    out=dv, in_=v_ones.rearrange("p a h q -> p (a h q)"))
                nc.sync.dma_start(out=dbg_v_d.ap(), in_=dv)
                don = big.tile([P, N_IC * H * QKV], F32, tag="dbgon")
                for ic in range(N_IC):
                    nc.vector.tensor_copy(
                        out=don[:, ic * H * QKV:(ic + 1) * H * QKV],
                        in_=outN[ic].rearrange("p b c -> p (b c)"))
                nc.sync.dma_start(out=dbg_on_d.ap(), in_=don)

            # ---- final projection ----------------------------------------
            for ic in range(N_IC):
                otc = work.tile([P, P], BF16, tag="otc", name=f"otc_{ic}")
                nc.sync.dma_start_transpose(
                    otc, outN[ic].rearrange("p h q -> p (h q)"))
                psf = ps_a.tile([P, D], F32, tag="pss", name=f"psf_{ic}")
                nc.tensor.matmul(psf, otc, wo[:, 0, :], start=True, stop=False)
                nc.tensor.matmul(psf, otc, wo[:, 1, :], start=False, stop=True)
                fin = work.tile([P, D], F32, tag="fin")
                nc.vector.tensor_copy(out=fin, in_=psf)
                nc.sync.dma_start(out=out_d.ap()[ic * P:(ic + 1) * P, :],
                                  in_=fin)

    nc.compile()
    return nc


def _hi_lo(x):
    import ml_dtypes
    bf = ml_dtypes.bfloat16
    hi = x.astype(bf)
    lo = (x.astype(np.float32) - hi.astype(np.float32)).astype(bf)
    return np.ascontiguousarray(hi), np.ascontiguousarray(lo)


def _make_in_maps(inputs):
    import ml_dtypes
    bf = ml_dtypes.bfloat16
    h = np.asarray(inputs["h"], dtype=np.float32)
    sc = np.asarray(inputs["same_cluster"])
    Wa = np.asarray(inputs["Wa"], dtype=np.float32)
    Wv = np.asarray(inputs["Wv"], dtype=np.float32)
    Wo = np.asarray(inputs["Wo"], dtype=np.float32)

    wai_hi, wai_lo = _hi_lo(Wa[:D])
    waj_hi, waj_lo = _hi_lo(Wa[D:2 * D])
    wv_hi, wv_lo = _hi_lo(Wv)
    wo_hi, wo_lo = _hi_lo(Wo)
    we_row = np.ascontiguousarray(Wa[2 * D:2 * D + 1, :])
    eye = np.eye(P, dtype=np.float32)
    ones = np.ones((1, H * N), dtype=np.float32)

    sc_bf = sc.astype(bf)
    h_hi_b = {}
    h_lo_b = {}
    for b in range(B):
        h_hi_b[b], h_lo_b[b] = _hi_lo(h[b])

    in_maps = []
    for c in range(N_CORES):
        b = c // 2
        r0 = (c % 2) * ROWS
        in_maps.append({
            "h_hi": h_hi_b[b],
            "h_lo": h_lo_b[b],
            "hr_hi": np.ascontiguousarray(h_hi_b[b][r0:r0 + ROWS]),
            "hr_lo": np.ascontiguousarray(h_lo_b[b][r0:r0 + ROWS]),
            "sc_bf": np.ascontiguousarray(sc_bf[b, r0:r0 + ROWS, :]),
            "wai_hi": wai_hi, "wai_lo": wai_lo,
            "waj_hi": waj_hi, "waj_lo": waj_lo,
            "wv_hi": wv_hi, "wv_lo": wv_lo,
            "wo_hi": wo_hi, "wo_lo": wo_lo,
            "we_row": we_row, "eye": eye, "ones": ones,
        })
    return in_maps


def _build_runner(nc):
    """Persistent jitted shard_map runner (avoids per-call retracing)."""
    import jax
    from jax.sharding import Mesh, PartitionSpec
    from jax.experimental.shard_map import shard_map
    from concourse.bass2jax import (
        _bass_exec_p, install_neuronx_cc_hook, partition_id_tensor,
    )

    install_neuronx_cc_hook()
    partition_name = nc.partition_id_tensor.name if nc.partition_id_tensor else None
    in_names, out_names, out_avals, zero_shapes = [], [], [], []
    for alloc in nc.m.functions[0].allocations:
        if not isinstance(alloc, mybir.MemoryLocationSet):
            continue
        name = alloc.memorylocations[0].name
        if alloc.kind == "ExternalInput":
            if name != partition_name:
                in_names.append(name)
        elif alloc.kind == "ExternalOutput":
            out_names.append(name)
            shape = tuple(alloc.tensor_shape)
            dtype = mybir.dt.np(alloc.dtype)
            out_avals.append(jax.core.ShapedArray(shape, dtype))
            zero_shapes.append((shape, dtype))
    n_params = len(in_names)
    all_in_names = list(in_names) + list(out_names)
    if partition_name is not None:
        all_in_names.append(partition_name)

    def _body(*args):
        operands = list(args)
        if partition_name is not None:
            operands.append(partition_id_tensor())
        outs = _bass_exec_p.bind(
            *operands,
            out_avals=tuple(out_avals),
            in_names=tuple(all_in_names),
            out_names=tuple(out_names),
            lowering_input_output_aliases=(),
            sim_require_finite=True,
            sim_require_nnan=True,
            nc=nc,
        )
        return tuple(outs)

    devices = jax.devices()[:N_CORES]
    mesh = Mesh(np.asarray(devices), ("core",))
    in_specs = (PartitionSpec("core"),) * (n_params + len(out_names))
    out_specs = (PartitionSpec("core"),) * len(out_names)
    fn = jax.jit(
        shard_map(_body, mesh=mesh, in_specs=in_specs, out_specs=out_specs,
                  check_rep=False),
        donate_argnums=tuple(range(n_params, n_params + len(out_names))),
        keep_unused=True,
    )
    return fn, in_names, out_names, zero_shapes


def kernel(h, same_cluster, Wa, Wv, Wo, local_only):
    local_only = int(local_only)
    key = ("prog", local_only)
    if key not in _cache:
        _cache[key] = _build_program(local_only)
    nc = _cache[key]

    in_maps = _make_in_maps({
        "h": h, "same_cluster": same_cluster, "Wa": Wa, "Wv": Wv, "Wo": Wo,
    })

    try:
        rkey = ("runner", local_only)
        if rkey not in _cache:
            _cache[rkey] = _build_runner(nc)
        fn, in_names, out_names, zero_shapes = _cache[rkey]
        concat_in = [
            np.concatenate([np.asarray(in_maps[c][nm]) for c in range(N_CORES)],
                           axis=0)
            for nm in in_names
        ]
        concat_zeros = [
            np.zeros((N_CORES * s[0], *s[1:]), dt) for s, dt in zero_shapes
        ]
        out_arrs = fn(*concat_in, *concat_zeros)
        res_per_core = np.asarray(out_arrs[out_names.index("out_rows")]).reshape(
            N_CORES, ROWS, D
        )
    except Exception:
        res = run_bass_kernel_spmd(nc, in_maps, list(range(N_CORES)))
        res_per_core = np.stack(
            [res.results[c]["out_rows"] for c in range(N_CORES)]
        )

    out = np.empty((B, N, D), dtype=np.float32)
    for c in range(N_CORES):
        b = c // 2
        r0 = (c % 2) * ROWS
        out[b, r0:r0 + ROWS, :] = res_per_core[c]
    return out


if __name__ == "__main__":
    rng = np.random.default_rng(0)
    h = rng.standard_normal((B, N, D), dtype=np.float32)
    sc = rng.integers(0, 2, (B, N, N)).astype(bool)
    Wa = rng.standard_normal((2 * D + 1, H), dtype=np.float32) / np.sqrt(2 * D + 1)
    Wv = rng.standard_normal((D, H * QKV), dtype=np.float32) / np.sqrt(D)
    Wo = rng.standard_normal((H * QKV, D), dtype=np.float32) / np.sqrt(H * QKV)

    out = kernel(h=h, same_cluster=sc, Wa=Wa, Wv=Wv, Wo=Wo, local_only=0)

    Wa_i, Wa_j, w_e = Wa[:D], Wa[D:2 * D], Wa[2 * D]
    s_i = h @ Wa_i
    s_j = h @ Wa_j
    scores = (s_i[:, :, None, :] + s_j[:, None, :, :]
              + sc.astype(np.float32)[..., None] * w_e)
    scores = np.where(scores > 0, scores, NEG_SLOPE * scores)
    scores = np.moveaxis(scores, -1, 1)
    scores = scores - scores.max(axis=-1, keepdims=True)
    e = np.exp(scores)
    alpha = e / e.sum(axis=-1, keepdims=True)
    v = (h @ Wv).reshape(B, N, H, QKV).transpose(0, 2, 1, 3)
    o = np.einsum('bhij,bhjd->bhid', alpha, v)
    o = o.transpose(0, 2, 1, 3).reshape(B, N, H * QKV)
    expected = o @ Wo

    err = np.abs(out - expected)
    rel = np.linalg.norm(out - expected) / np.linalg.norm(expected)
    print(f"rel_err(norm)={rel:.3e} max_abs={err.max():.3e}")
